# revision 21
# baseline (speedup 1.0000x reference)
"""AgentAttention fused Trainium2 kernel (8-core data-parallel over batch).

Reference computation (per batch, n=3136=56x56, c=384, 8 heads, 16 agents):
    q = x @ Wq.T ; k = x @ Wkv.T
    agent_q = pool(q); agent_k = pool(k)            # 4x4 adaptive avg pool
    A = (agent_q @ w_g) * scale; G = sum(A * agent_q, 1)
    agent_new = (G * agent_k) @ Wp.T + bp + agent_q
    attn = softmax(scale * q_h @ agent_new_h.T)     # per head
    out = (attn @ agent_k_h) -> concat -> @ Wp.T + bp

Key algebraic fusions used here:
  1. pooling commutes with the linear projections:
         agent_q = pool(x) @ Wq.T, agent_k = pool(x) @ Wkv.T
     so the full k = x @ Wkv.T is NEVER computed.
  2. logits fold Wq:  logitsT = (anbd.T @ Wq) @ xT = M_bdT.T @ xT where
     anbd is the block-diagonal [c, h*a] layout of scale*agent_new^T.
     So the full q is never computed either.
  3. softmax without max subtraction (logits are O(0.1) for this problem).
  4. per-head softmax sums via a matmul with a 0/1 head-selector, and the
     1/sum broadcast back to 128 partitions via a second tiny matmul.
  5. output projection folded into the value matrix:
         out = attnT.T @ akp + bp,  akp = akbd.T @ Wp.T
     and since softmax rows sum to 1 per head, bp is folded as akp += bp/8
     via a K=1 rank-1 matmul. attn is produced directly transposed.

x is pre-cast to bf16 on the host so it can be loaded with the hardware
transpose DMA (2-byte dtypes only); everything else stays fp32/f32r.

DMA structure note: DMA/TensorReduce instructions have a single HW
sync-wait slot, so the program uses exactly 3 transpose DMAs (all batches
at once, issued before any other DMA), 3 constant-blob loads, and one
merged store per n-chunk to keep each DMA's dependency count at <= 1.
"""

import numpy as np
import ml_dtypes
from contextlib import ExitStack

import concourse.bass as bass
import concourse.bacc as bacc
import concourse.mybir as mybir
import concourse.tile as tile
from concourse.bass_utils import run_bass_kernel_spmd

NCORES = 8
B_FULL = 32
BPC = B_FULL // NCORES   # 4 batches per core
N = 3136                 # 56*56
C = 384
P = 128
CH = C // P              # 3 c-chunks
NCHUNK = 448
NCH = N // NCHUNK        # 7 n-chunks per batch
HEADS = 8
D = C // HEADS           # 48
A = 16                   # agents
HA = HEADS * A           # 128
SCALE = float(D) ** -0.5
POOLN = 196.0            # 14*14 elements per pooling block

F32 = mybir.dt.float32
F32R = mybir.dt.float32r
BF16 = mybir.dt.bfloat16

ADD = mybir.AluOpType.add
MULT = mybir.AluOpType.mult

# constant blob column layouts (per 128-partition row, in elements)
#   f32 blob : wqt(CH*C) | wkvt(CH*C) | wg(CH) | bpv(CH) | bdmask(CH*HA)
#   f32r blob: wpt(CH*C) | bp8(C) | hsel(HEADS) | hselt(HA) | ones1(HA)
#   bf16 blob: wq16(CH*C)
_F32_SECTS = [("wqt", CH * C), ("wkvt", CH * C), ("wg", CH), ("bpv", CH),
              ("bdmask", CH * HA)]
_F32R_SECTS = [("wpt", CH * C), ("bp8", C), ("hsel", HEADS),
               ("hselt", HA), ("ones1", HA)]
F32_COLS = sum(n for _, n in _F32_SECTS)
F32R_COLS = sum(n for _, n in _F32R_SECTS)


def _offsets(sects):
    out, o = {}, 0
    for name, n in sects:
        out[name] = (o, n)
        o += n
    return out


F32_OFF = _offsets(_F32_SECTS)
F32R_OFF = _offsets(_F32R_SECTS)


def build_nc(stage=None):
    import os
    if stage is None:
        stage = int(os.environ.get("KSTAGE", "1000"))
    # Bacc (not plain Bass): its compile() runs generate_event_semaphores,
    # which splits multi-wait instructions (TRN2 allows 1 wait per inst).
    nc = bacc.Bacc(None, target_bir_lowering=False, debug=False)

    x16 = nc.dram_tensor("x16", [BPC, N, C], BF16, kind="ExternalInput")
    blob16 = nc.dram_tensor("blob16", [P, CH * C], BF16, kind="ExternalInput")
    blobf = nc.dram_tensor("blobf", [P, F32_COLS], F32, kind="ExternalInput")
    blobr = nc.dram_tensor("blobr", [P, F32R_COLS], F32R,
                           kind="ExternalInput")
    out = nc.dram_tensor("out", [BPC, N, C], F32, kind="ExternalOutput")

    with tile.TileContext(nc) as tc, ExitStack() as ctx:
        consts = ctx.enter_context(tc.tile_pool(name="consts", bufs=1))
        xtp = ctx.enter_context(tc.tile_pool(name="xt", bufs=1))
        agents = ctx.enter_context(tc.tile_pool(name="agents", bufs=BPC))
        chunkp = ctx.enter_context(tc.tile_pool(name="chunk", bufs=3))
        outp = ctx.enter_context(tc.tile_pool(name="outp", bufs=3))
        psA = ctx.enter_context(
            tc.tile_pool(name="psA", bufs=2, space=bass.MemorySpace.PSUM))
        psL = ctx.enter_context(
            tc.tile_pool(name="psL", bufs=2, space=bass.MemorySpace.PSUM))
        psS = ctx.enter_context(
            tc.tile_pool(name="psS", bufs=1, space=bass.MemorySpace.PSUM))
        psB = ctx.enter_context(
            tc.tile_pool(name="psB", bufs=1, space=bass.MemorySpace.PSUM))
        psF = ctx.enter_context(
            tc.tile_pool(name="psF", bufs=2, space=bass.MemorySpace.PSUM))

        # ---- transpose loads first (XPOSE insts must not follow any copy) ----
        # one transpose per c-chunk covering ALL batches: in [BPC*N, 128]
        xt_all = xtp.tile([P, CH, BPC * N], BF16, tag="xt")
        xsrc = x16[:].rearrange("b n c -> (b n) c")
        for ci in range(CH):
            nc.sync.dma_start_transpose(
                xt_all[:, ci, :], xsrc[:, ci * P:(ci + 1) * P])

        # ---- constant blobs (3 copies) ----
        sb16 = consts.tile([P, CH * C], BF16, tag="sb16")
        sbf = consts.tile([P, F32_COLS], F32, tag="sbf")
        sbr = consts.tile([P, F32R_COLS], F32R, tag="sbr")
        nc.sync.dma_start(sb16[:], blob16[:])
        nc.sync.dma_start(sbf[:], blobf[:])
        nc.sync.dma_start(sbr[:], blobr[:])

        def fview(name, nmid):
            o, n = F32_OFF[name]
            v = sbf[:, o:o + n]
            return v.rearrange("p (a b) -> p a b", a=nmid) if nmid else v

        def rview(name, nmid):
            o, n = F32R_OFF[name]
            v = sbr[:, o:o + n]
            return v.rearrange("p (a b) -> p a b", a=nmid) if nmid else v

        wq_sb = sb16[:].rearrange("p (ch c) -> p ch c", ch=CH)
        wqt_sb = fview("wqt", CH)
        wkvt_sb = fview("wkvt", CH)
        wg_sb = fview("wg", CH)
        bp_sb = fview("bpv", CH)
        bdm_sb = fview("bdmask", CH)
        wpt_sb = rview("wpt", CH)
        bp8_sb = rview("bp8", 0)[0:1, :]
        hsel_sb = rview("hsel", 0)
        hselt_sb = rview("hselt", 0)[0:HEADS, :]
        ones1_sb = rview("ones1", 0)[0:1, :]

        for b in range(BPC):
            if stage < 2:
                break
            xt = xt_all[:, :, b * N:(b + 1) * N]

            # ---- pooling: xpT[c, a] = block sums (1/196 folded into wqt/wkvt)
            # two passes, pass1 for all chunks first: TensorReduce has a single
            # sync-wait slot, so each reduce may carry only one dependency
            # (pass1: the transpose DMA; pass2: the pass1 DVE op).
            xpT = agents.tile([P, CH, A], F32, tag="xpT")
            ptmps = [agents.tile([P, 224], F32, tag=f"ptmp{ci}",
                                 name=f"ptmp{ci}") for ci in range(CH)]
            for ci in range(CH):
                # n = (14*hi+hr)*56 + 14*wi+wr ; merge (hi,hr) -> rows of 56
                v1 = xt[:, ci, :].rearrange("p (hh wi wr) -> p hh wi wr",
                                            hh=56, wi=4, wr=14)
                nc.vector.tensor_reduce(
                    ptmps[ci][:].rearrange("p (hh wi) -> p hh wi", hh=56),
                    v1, axis=mybir.AxisListType.X, op=ADD)
            for ci in range(CH):
                v2 = ptmps[ci][:].rearrange("p (hi hr wi) -> p hi wi hr",
                                            hi=4, hr=14, wi=4)
                nc.vector.tensor_reduce(
                    xpT[:, ci, :].rearrange("p (hi wi) -> p hi wi", hi=4),
                    v2, axis=mybir.AxisListType.X, op=ADD)

            if stage < 3:
                continue
            # ---- agent projections: aqT = (Wq.T/196).T @ xpT etc. ----
            aq_ps = psA.tile([P, CH, A], F32, tag="ag")
            for co in range(CH):
                for ci in range(CH):
                    nc.tensor.matmul(
                        aq_ps[:, co, :], wqt_sb[:, ci, co * P:(co + 1) * P],
                        xpT[:, ci, :], start=(ci == 0), stop=(ci == CH - 1))
            aqT = agents.tile([P, CH, A], F32, tag="aqT")
            nc.vector.tensor_copy(aqT[:], aq_ps[:])

            ak_ps = psA.tile([P, CH, A], F32, tag="ag")
            for co in range(CH):
                for ci in range(CH):
                    nc.tensor.matmul(
                        ak_ps[:, co, :], wkvt_sb[:, ci, co * P:(co + 1) * P],
                        xpT[:, ci, :], start=(ci == 0), stop=(ci == CH - 1))
            akT = agents.tile([P, CH, A], F32, tag="akT")
            nc.vector.tensor_copy(akT[:], ak_ps[:])

            if stage < 4:
                continue
            # ---- gating: A = (agent_q @ w_g)*scale ; G = sum_a A*agent_q ----
            a_ps = psA.tile([1, A], F32, tag="ag")
            for ci in range(CH):
                nc.tensor.matmul(a_ps[:], wg_sb[:, ci, :], aqT[:, ci, :],
                                 start=(ci == 0), stop=(ci == CH - 1))
            if stage < 41:
                continue
            a_sb = agents.tile([1, A], F32, tag="a_sb")
            nc.vector.tensor_scalar_mul(a_sb[:], a_ps[:], SCALE)
            if stage < 42:
                continue
            # broadcast A row to 128 partitions via rank-1 matmul
            ar_ps = psA.tile([P, A], F32, tag="ag")
            nc.tensor.matmul(ar_ps[:], ones1_sb[:].bitcast(F32), a_sb[:],
                             start=True, stop=True)
            if stage < 43:
                continue
            gvec = agents.tile([P, CH, 1], F32, tag="gvec")
            gscr = agents.tile([P, CH, A], F32, tag="gscr")
            for ci in range(CH):
                nc.vector.tensor_mul(gscr[:, ci, :], aqT[:, ci, :], ar_ps[:])
            for ci in range(CH):
                nc.vector.tensor_reduce(gvec[:, ci, :], gscr[:, ci, :],
                                        axis=mybir.AxisListType.X, op=ADD)

            if stage < 50:
                continue
            # ---- agent_new^T = Wp @ (G*akT) + bp + aqT, scaled ----
            if stage < 44:
                continue
            gkT = agents.tile([P, CH, A], F32, tag="gkT")
            for ci in range(CH):
                nc.vector.tensor_scalar_mul(gkT[:, ci, :], akT[:, ci, :],
                                            gvec[:, ci, :])
            an_ps = psA.tile([P, CH, A], F32, tag="ag")
            for co in range(CH):
                for ci in range(CH):
                    nc.tensor.matmul(
                        an_ps[:, co, :],
                        wpt_sb[:, ci, co * P:(co + 1) * P].bitcast(F32),
                        gkT[:, ci, :], start=(ci == 0), stop=(ci == CH - 1))
            an_st = agents.tile([P, CH, A], F32, tag="an_st")
            for ci in range(CH):
                nc.vector.tensor_add(an_st[:, ci, :], an_ps[:, ci, :],
                                     aqT[:, ci, :])
                nc.vector.tensor_scalar(an_st[:, ci, :], an_st[:, ci, :],
                                        bp_sb[:, ci, :], SCALE, ADD, MULT)

            if stage < 60:
                continue
            # ---- block-diagonals via mask-multiply (whole tile written) ----
            anbd = agents.tile([P, CH, HA], BF16, tag="anbd")
            akbd = agents.tile([P, CH, HA], F32R, tag="akbd")
            mbdT = agents.tile([P, CH, HA], BF16, tag="mbdT")
            for ci in range(CH):
                src_an = an_st[:, ci, :].unsqueeze(1).broadcast_to(
                    [P, HEADS, A])
                src_ak = akT[:, ci, :].unsqueeze(1).broadcast_to(
                    [P, HEADS, A])
                msk = bdm_sb[:, ci, :].rearrange("p (h a) -> p h a", h=HEADS)
                nc.vector.tensor_mul(
                    anbd[:, ci, :].rearrange("p (h a) -> p h a", h=HEADS),
                    src_an, msk)
                nc.vector.tensor_mul(
                    akbd[:, ci, :].rearrange("p (h a) -> p h a", h=HEADS),
                    src_ak, msk)

            if stage < 70:
                continue
            # ---- M_bdT[i, ha] = sum_o Wq[o,i] * anbd[o,ha]  (bf16) ----
            for ci in range(CH):
                mbd_ps = psA.tile([P, HA], F32, tag="ag")
                for co in range(CH):
                    nc.tensor.matmul(mbd_ps[:],
                                     wq_sb[:, co, ci * P:(ci + 1) * P],
                                     anbd[:, co, :],
                                     start=(co == 0), stop=(co == CH - 1))
                nc.vector.tensor_copy(mbdT[:, ci, :], mbd_ps[:])

            if stage < 80:
                continue
            # ---- akp[ha, o] = akbd.T @ Wp.T + bp/8 (rank-1 bias matmul) ----
            akp_ps = psF.tile([P, C], F32, tag="fp")
            for ci in range(CH):
                nc.tensor.matmul(akp_ps[:], akbd[:, ci, :],
                                 wpt_sb[:, ci, :],
                                 start=(ci == 0), stop=False)
            nc.tensor.matmul(akp_ps[:], ones1_sb[:],
                             bp8_sb[:], start=False, stop=True)
            akp_sb = agents.tile([P, C], F32R, tag="akp")
            nc.vector.tensor_copy(akp_sb[:], akp_ps[:])

            # ---- main attention loop over n-chunks ----
            if stage < 90:
                continue
            for t in range(NCH):
                ns = slice(t * NCHUNK, (t + 1) * NCHUNK)
                l_ps = psL.tile([P, NCHUNK], F32, tag="l")
                for ci in range(CH):
                    nc.tensor.matmul(l_ps[:], mbdT[:, ci, :], xt[:, ci, ns],
                                     start=(ci == 0), stop=(ci == CH - 1))
                expT = chunkp.tile([P, NCHUNK], F32R, tag="exp")
                nc.scalar.activation(expT[:], l_ps[:],
                                     mybir.ActivationFunctionType.Exp)
                s_ps = psS.tile([HEADS, NCHUNK], F32, tag="s")
                nc.tensor.matmul(s_ps[:], hsel_sb[:], expT[:],
                                 start=True, stop=True)
                rec = chunkp.tile([HEADS, NCHUNK], F32R, tag="rec")
                with nc.allow_low_precision(reason="f32r softmax denom"):
                    nc.vector.reciprocal(rec[:], s_ps[:])
                b_ps = psB.tile([P, NCHUNK], F32, tag="b")
                nc.tensor.matmul(b_ps[:], hselt_sb[:], rec[:],
                                 start=True, stop=True)
                attnT = chunkp.tile([P, NCHUNK], F32R, tag="attn")
                nc.vector.tensor_mul(attnT[:], expT[:].bitcast(F32), b_ps[:])

                ob = outp.tile([112, 4, C], F32, tag="ob")
                for s in range(4):
                    c0 = s * 112
                    f_ps = psF.tile([P, C], F32, tag="fp")
                    nc.tensor.matmul(f_ps[:112, :],
                                     attnT[:, c0:c0 + 112], akp_sb[:],
                                     start=True, stop=True)
                    nc.vector.tensor_copy(ob[:, s, :], f_ps[:112, :])
                # one merged store per chunk: row n0+s*128+p <- ob[p, s, :]
                if stage >= 100:
                    dst = out[b, t * NCHUNK:(t + 1) * NCHUNK, :].rearrange(
                        "(s p) c -> p s c", s=4)
                    nc.scalar.dma_start(dst, ob[:])

    # Bacc.finalize -> compile(): alloc_regs + generate_event_semaphores
    # (multi-wait split). The PJRT run path does not finalize for us.
    nc.finalize()
    return nc


_CACHE = {}


def _get_nc():
    if "nc" not in _CACHE:
        _CACHE["nc"] = build_nc()
    return _CACHE["nc"]


def _make_const_inputs(Wq, Wkv, w_g, Wp, bp):
    Wq = np.ascontiguousarray(np.asarray(Wq, np.float32))
    Wkv = np.ascontiguousarray(np.asarray(Wkv, np.float32))
    Wp = np.ascontiguousarray(np.asarray(Wp, np.float32))
    w_g = np.ascontiguousarray(np.asarray(w_g, np.float32))
    bp = np.ascontiguousarray(np.asarray(bp, np.float32))

    def chunked(m):  # [CH*P, X] -> [P, CH*X] partition-major layout
        x = m.reshape(CH, P, -1).transpose(1, 0, 2)
        return np.ascontiguousarray(x).reshape(P, -1)

    hsel = np.zeros((HA, HEADS), np.float32)
    hsel[np.arange(HA), np.arange(HA) // A] = 1.0
    bdmask = np.zeros((C, HA), np.float32)
    for c in range(C):
        h = c // D
        bdmask[c, h * A:(h + 1) * A] = 1.0

    fparts = {
        "wqt": chunked(Wq.T / POOLN),
        "wkvt": chunked(Wkv.T / POOLN),
        "wg": chunked(w_g),
        "bpv": chunked(bp.reshape(C, 1)),
        "bdmask": chunked(bdmask),
    }
    blobf = np.concatenate([fparts[n] for n, _ in _F32_SECTS], axis=1)

    rp = {"wpt": chunked(Wp.T)}
    z = np.zeros((P, C), np.float32)
    z[0] = bp / HEADS
    rp["bp8"] = z
    rp["hsel"] = hsel
    z = np.zeros((P, HA), np.float32)
    z[:HEADS] = hsel.T
    rp["hselt"] = z
    z = np.zeros((P, HA), np.float32)
    z[0] = 1.0
    rp["ones1"] = z
    blobr = np.concatenate([rp[n] for n, _ in _F32R_SECTS], axis=1)

    blob16 = chunked(Wq).astype(ml_dtypes.bfloat16)
    return {"blob16": blob16, "blobf": blobf, "blobr": blobr}


def kernel(x, H=56, W=56, Wq=None, Wkv=None, w_g=None, Wp=None, bp=None,
           _trace=False, _trace_kwargs=None):
    x = np.asarray(x)
    assert x.shape == (B_FULL, N, C), x.shape
    x16 = np.asarray(x, np.float32).astype(ml_dtypes.bfloat16)

    consts = _make_const_inputs(Wq, Wkv, w_g, Wp, bp)
    in_maps = []
    for c in range(NCORES):
        m = dict(consts)
        m["x16"] = np.ascontiguousarray(x16[c * BPC:(c + 1) * BPC])
        in_maps.append(m)

    nc = _get_nc()
    res = run_bass_kernel_spmd(nc, in_maps, list(range(NCORES)),
                               trace=_trace, **(_trace_kwargs or {}))
    outs = np.concatenate([res.results[c]["out"] for c in range(NCORES)],
                          axis=0)
    if _trace:
        return outs.astype(np.float32, copy=False), res
    return outs.astype(np.float32, copy=False)


# revision 22
# speedup vs baseline: 30622.6459x; 30622.6459x over previous
"""AgentAttention fused Trainium2 kernel (8-core data-parallel over batch).

Reference computation (per batch, n=3136=56x56, c=384, 8 heads, 16 agents):
    q = x @ Wq.T ; k = x @ Wkv.T
    agent_q = pool(q); agent_k = pool(k)            # 4x4 adaptive avg pool
    A = (agent_q @ w_g) * scale; G = sum(A * agent_q, 1)
    agent_new = (G * agent_k) @ Wp.T + bp + agent_q
    attn = softmax(scale * q_h @ agent_new_h.T)     # per head
    out = (attn @ agent_k_h) -> concat -> @ Wp.T + bp

Key algebraic fusions used here:
  1. pooling commutes with the linear projections:
         agent_q = pool(x) @ Wq.T, agent_k = pool(x) @ Wkv.T
     so the full k = x @ Wkv.T is NEVER computed.
  2. logits fold Wq:  logitsT = (anbd.T @ Wq) @ xT = M_bdT.T @ xT where
     anbd is the block-diagonal [c, h*a] layout of scale*agent_new^T.
     So the full q is never computed either.
  3. softmax without max subtraction (logits are O(0.1) for this problem).
  4. per-head softmax sums via a matmul with a 0/1 head-selector, and the
     1/sum broadcast back to 128 partitions via a second tiny matmul.
  5. output projection folded into the value matrix:
         out = attnT.T @ akp + bp,  akp = akbd.T @ Wp.T
     and since softmax rows sum to 1 per head, bp is folded as akp += bp/8
     via a K=1 rank-1 matmul. attn is produced directly transposed.

x is pre-cast to bf16 on the host so it can be loaded with the hardware
transpose DMA (2-byte dtypes only); everything else stays fp32/f32r.

DMA structure note: DMA/TensorReduce instructions have a single HW
sync-wait slot, so the program uses exactly 3 transpose DMAs (all batches
at once, issued before any other DMA), 3 constant-blob loads, and one
merged store per n-chunk to keep each DMA's dependency count at <= 1.
"""

import numpy as np
import ml_dtypes
from contextlib import ExitStack

import concourse.bass as bass
import concourse.bacc as bacc
import concourse.mybir as mybir
import concourse.tile as tile
from concourse.bass_utils import run_bass_kernel_spmd

NCORES = 8
B_FULL = 32
BPC = B_FULL // NCORES   # 4 batches per core
N = 3136                 # 56*56
C = 384
P = 128
CH = C // P              # 3 c-chunks
NCHUNK = 448
NCH = N // NCHUNK        # 7 n-chunks per batch
HEADS = 8
D = C // HEADS           # 48
A = 16                   # agents
HA = HEADS * A           # 128
SCALE = float(D) ** -0.5
POOLN = 196.0            # 14*14 elements per pooling block

F32 = mybir.dt.float32
F32R = mybir.dt.float32r
BF16 = mybir.dt.bfloat16

ADD = mybir.AluOpType.add
MULT = mybir.AluOpType.mult

# constant blob column layouts (per 128-partition row, in elements)
#   f32 blob : wqt(CH*C) | wkvt(CH*C) | wg(CH) | bpv(CH) | bdmask(CH*HA)
#   f32r blob: wpt(CH*C) | bp8(C) | hsel(HEADS) | hselt(HA) | ones1(HA)
#   bf16 blob: wq16(CH*C)
_F32_SECTS = [("wqt", CH * C), ("wkvt", CH * C), ("wg", CH), ("bpv", CH),
              ("bdmask", CH * HA)]
_F32R_SECTS = [("wpt", CH * C), ("bp8", C), ("hsel", HEADS),
               ("hselt", HA), ("ones1", HA)]
F32_COLS = sum(n for _, n in _F32_SECTS)
F32R_COLS = sum(n for _, n in _F32R_SECTS)


def _offsets(sects):
    out, o = {}, 0
    for name, n in sects:
        out[name] = (o, n)
        o += n
    return out


F32_OFF = _offsets(_F32_SECTS)
F32R_OFF = _offsets(_F32R_SECTS)


def build_nc(stage=None):
    import os
    if stage is None:
        stage = int(os.environ.get("KSTAGE", "1000"))
    # Bacc (not plain Bass): its compile() runs generate_event_semaphores,
    # which splits multi-wait instructions (TRN2 allows 1 wait per inst).
    nc = bacc.Bacc(None, target_bir_lowering=False, debug=False)

    x16 = nc.dram_tensor("x16", [BPC, N, C], BF16, kind="ExternalInput")
    blob16 = nc.dram_tensor("blob16", [P, CH * C], BF16, kind="ExternalInput")
    blobf = nc.dram_tensor("blobf", [P, F32_COLS], F32, kind="ExternalInput")
    blobr = nc.dram_tensor("blobr", [P, F32R_COLS], F32R,
                           kind="ExternalInput")
    out = nc.dram_tensor("out", [BPC, N, C], F32, kind="ExternalOutput")

    with tile.TileContext(nc) as tc, ExitStack() as ctx:
        consts = ctx.enter_context(tc.tile_pool(name="consts", bufs=1))
        xtp = ctx.enter_context(tc.tile_pool(name="xt", bufs=1))
        agents = ctx.enter_context(tc.tile_pool(name="agents", bufs=BPC))
        chunkp = ctx.enter_context(tc.tile_pool(name="chunk", bufs=3))
        outp = ctx.enter_context(tc.tile_pool(name="outp", bufs=3))
        psA = ctx.enter_context(
            tc.tile_pool(name="psA", bufs=2, space=bass.MemorySpace.PSUM))
        psL = ctx.enter_context(
            tc.tile_pool(name="psL", bufs=2, space=bass.MemorySpace.PSUM))
        psS = ctx.enter_context(
            tc.tile_pool(name="psS", bufs=1, space=bass.MemorySpace.PSUM))
        psB = ctx.enter_context(
            tc.tile_pool(name="psB", bufs=1, space=bass.MemorySpace.PSUM))
        psF = ctx.enter_context(
            tc.tile_pool(name="psF", bufs=2, space=bass.MemorySpace.PSUM))

        # ---- transpose loads first (before any regular copy: Bacc's
        # event-semaphore pass splits multi-wait instructions, but keeping
        # the xposes up front still minimizes xbar-mode transitions).
        # Per-batch, batch-major: batch 0's chunks land first so compute
        # starts ~6us in rather than after the full 9.6MB transpose.
        xt_all = xtp.tile([P, CH, BPC * N], BF16, tag="xt")
        for b in range(BPC):
            for ci in range(CH):
                nc.sync.dma_start_transpose(
                    xt_all[:, ci, b * N:(b + 1) * N],
                    x16[b, :, ci * P:(ci + 1) * P])

        # ---- constant blobs (3 copies) ----
        sb16 = consts.tile([P, CH * C], BF16, tag="sb16")
        sbf = consts.tile([P, F32_COLS], F32, tag="sbf")
        sbr = consts.tile([P, F32R_COLS], F32R, tag="sbr")
        nc.sync.dma_start(sb16[:], blob16[:])
        nc.sync.dma_start(sbf[:], blobf[:])
        nc.sync.dma_start(sbr[:], blobr[:])

        def fview(name, nmid):
            o, n = F32_OFF[name]
            v = sbf[:, o:o + n]
            return v.rearrange("p (a b) -> p a b", a=nmid) if nmid else v

        def rview(name, nmid):
            o, n = F32R_OFF[name]
            v = sbr[:, o:o + n]
            return v.rearrange("p (a b) -> p a b", a=nmid) if nmid else v

        wq_sb = sb16[:].rearrange("p (ch c) -> p ch c", ch=CH)
        wqt_sb = fview("wqt", CH)
        wkvt_sb = fview("wkvt", CH)
        wg_sb = fview("wg", CH)
        bp_sb = fview("bpv", CH)
        bdm_sb = fview("bdmask", CH)
        wpt_sb = rview("wpt", CH)
        bp8_sb = rview("bp8", 0)[0:1, :]
        hsel_sb = rview("hsel", 0)
        hselt_sb = rview("hselt", 0)[0:HEADS, :]
        ones1_sb = rview("ones1", 0)[0:1, :]

        for b in range(BPC):
            if stage < 2:
                break
            xt = xt_all[:, :, b * N:(b + 1) * N]

            # ---- pooling: xpT[c, a] = block sums (1/196 folded into wqt/wkvt)
            # two passes, pass1 for all chunks first: TensorReduce has a single
            # sync-wait slot, so each reduce may carry only one dependency
            # (pass1: the transpose DMA; pass2: the pass1 DVE op).
            xpT = agents.tile([P, CH, A], F32, tag="xpT")
            ptmps = [agents.tile([P, 224], F32, tag=f"ptmp{ci}",
                                 name=f"ptmp{ci}") for ci in range(CH)]
            for ci in range(CH):
                # n = (14*hi+hr)*56 + 14*wi+wr ; merge (hi,hr) -> rows of 56
                v1 = xt[:, ci, :].rearrange("p (hh wi wr) -> p hh wi wr",
                                            hh=56, wi=4, wr=14)
                nc.vector.tensor_reduce(
                    ptmps[ci][:].rearrange("p (hh wi) -> p hh wi", hh=56),
                    v1, axis=mybir.AxisListType.X, op=ADD)
            for ci in range(CH):
                v2 = ptmps[ci][:].rearrange("p (hi hr wi) -> p hi wi hr",
                                            hi=4, hr=14, wi=4)
                nc.vector.tensor_reduce(
                    xpT[:, ci, :].rearrange("p (hi wi) -> p hi wi", hi=4),
                    v2, axis=mybir.AxisListType.X, op=ADD)

            if stage < 3:
                continue
            # ---- agent projections: aqT = (Wq.T/196).T @ xpT etc. ----
            aq_ps = psA.tile([P, CH, A], F32, tag="ag")
            for co in range(CH):
                for ci in range(CH):
                    nc.tensor.matmul(
                        aq_ps[:, co, :], wqt_sb[:, ci, co * P:(co + 1) * P],
                        xpT[:, ci, :], start=(ci == 0), stop=(ci == CH - 1))
            aqT = agents.tile([P, CH, A], F32, tag="aqT")
            nc.vector.tensor_copy(aqT[:], aq_ps[:])

            ak_ps = psA.tile([P, CH, A], F32, tag="ag")
            for co in range(CH):
                for ci in range(CH):
                    nc.tensor.matmul(
                        ak_ps[:, co, :], wkvt_sb[:, ci, co * P:(co + 1) * P],
                        xpT[:, ci, :], start=(ci == 0), stop=(ci == CH - 1))
            akT = agents.tile([P, CH, A], F32, tag="akT")
            nc.vector.tensor_copy(akT[:], ak_ps[:])

            if stage < 4:
                continue
            # ---- gating: A = (agent_q @ w_g)*scale ; G = sum_a A*agent_q ----
            a_ps = psA.tile([1, A], F32, tag="ag")
            for ci in range(CH):
                nc.tensor.matmul(a_ps[:], wg_sb[:, ci, :], aqT[:, ci, :],
                                 start=(ci == 0), stop=(ci == CH - 1))
            if stage < 41:
                continue
            a_sb = agents.tile([1, A], F32, tag="a_sb")
            nc.vector.tensor_scalar_mul(a_sb[:], a_ps[:], SCALE)
            if stage < 42:
                continue
            # broadcast A row to 128 partitions via rank-1 matmul
            ar_ps = psA.tile([P, A], F32, tag="ag")
            nc.tensor.matmul(ar_ps[:], ones1_sb[:].bitcast(F32), a_sb[:],
                             start=True, stop=True)
            if stage < 43:
                continue
            gvec = agents.tile([P, CH, 1], F32, tag="gvec")
            gscr = agents.tile([P, CH, A], F32, tag="gscr")
            for ci in range(CH):
                nc.vector.tensor_mul(gscr[:, ci, :], aqT[:, ci, :], ar_ps[:])
            for ci in range(CH):
                nc.vector.tensor_reduce(gvec[:, ci, :], gscr[:, ci, :],
                                        axis=mybir.AxisListType.X, op=ADD)

            if stage < 50:
                continue
            # ---- agent_new^T = Wp @ (G*akT) + bp + aqT, scaled ----
            if stage < 44:
                continue
            gkT = agents.tile([P, CH, A], F32, tag="gkT")
            for ci in range(CH):
                nc.vector.tensor_scalar_mul(gkT[:, ci, :], akT[:, ci, :],
                                            gvec[:, ci, :])
            an_ps = psA.tile([P, CH, A], F32, tag="ag")
            for co in range(CH):
                for ci in range(CH):
                    nc.tensor.matmul(
                        an_ps[:, co, :],
                        wpt_sb[:, ci, co * P:(co + 1) * P].bitcast(F32),
                        gkT[:, ci, :], start=(ci == 0), stop=(ci == CH - 1))
            an_st = agents.tile([P, CH, A], F32, tag="an_st")
            for ci in range(CH):
                nc.vector.tensor_add(an_st[:, ci, :], an_ps[:, ci, :],
                                     aqT[:, ci, :])
                nc.vector.tensor_scalar(an_st[:, ci, :], an_st[:, ci, :],
                                        bp_sb[:, ci, :], SCALE, ADD, MULT)

            if stage < 60:
                continue
            # ---- block-diagonals via mask-multiply (whole tile written) ----
            anbd = agents.tile([P, CH, HA], BF16, tag="anbd")
            akbd = agents.tile([P, CH, HA], F32R, tag="akbd")
            mbdT = agents.tile([P, CH, HA], BF16, tag="mbdT")
            for ci in range(CH):
                src_an = an_st[:, ci, :].unsqueeze(1).broadcast_to(
                    [P, HEADS, A])
                src_ak = akT[:, ci, :].unsqueeze(1).broadcast_to(
                    [P, HEADS, A])
                msk = bdm_sb[:, ci, :].rearrange("p (h a) -> p h a", h=HEADS)
                nc.vector.tensor_mul(
                    anbd[:, ci, :].rearrange("p (h a) -> p h a", h=HEADS),
                    src_an, msk)
                nc.vector.tensor_mul(
                    akbd[:, ci, :].rearrange("p (h a) -> p h a", h=HEADS),
                    src_ak, msk)

            if stage < 70:
                continue
            # ---- M_bdT[i, ha] = sum_o Wq[o,i] * anbd[o,ha]  (bf16) ----
            for ci in range(CH):
                mbd_ps = psA.tile([P, HA], F32, tag="ag")
                for co in range(CH):
                    nc.tensor.matmul(mbd_ps[:],
                                     wq_sb[:, co, ci * P:(ci + 1) * P],
                                     anbd[:, co, :],
                                     start=(co == 0), stop=(co == CH - 1))
                nc.vector.tensor_copy(mbdT[:, ci, :], mbd_ps[:])

            if stage < 80:
                continue
            # ---- akp[ha, o] = akbd.T @ Wp.T + bp/8 (rank-1 bias matmul) ----
            akp_ps = psF.tile([P, C], F32, tag="fp")
            for ci in range(CH):
                nc.tensor.matmul(akp_ps[:], akbd[:, ci, :],
                                 wpt_sb[:, ci, :],
                                 start=(ci == 0), stop=False)
            nc.tensor.matmul(akp_ps[:], ones1_sb[:],
                             bp8_sb[:], start=False, stop=True)
            akp_sb = agents.tile([P, C], F32R, tag="akp")
            nc.vector.tensor_copy(akp_sb[:], akp_ps[:])

            # ---- main attention loop over n-chunks ----
            if stage < 90:
                continue
            for t in range(NCH):
                ns = slice(t * NCHUNK, (t + 1) * NCHUNK)
                l_ps = psL.tile([P, NCHUNK], F32, tag="l")
                for ci in range(CH):
                    nc.tensor.matmul(l_ps[:], mbdT[:, ci, :], xt[:, ci, ns],
                                     start=(ci == 0), stop=(ci == CH - 1))
                expT = chunkp.tile([P, NCHUNK], F32R, tag="exp")
                nc.scalar.activation(expT[:], l_ps[:],
                                     mybir.ActivationFunctionType.Exp)
                s_ps = psS.tile([HEADS, NCHUNK], F32, tag="s")
                nc.tensor.matmul(s_ps[:], hsel_sb[:], expT[:],
                                 start=True, stop=True)
                rec = chunkp.tile([HEADS, NCHUNK], F32, tag="rec")
                nc.vector.reciprocal_approx_fast(rec[:], s_ps[:])
                b_ps = psB.tile([P, NCHUNK], F32, tag="b")
                nc.tensor.matmul(b_ps[:], hselt_sb[:].bitcast(F32), rec[:],
                                 start=True, stop=True)
                attnT = chunkp.tile([P, NCHUNK], F32R, tag="attn")
                nc.vector.tensor_mul(attnT[:], expT[:].bitcast(F32), b_ps[:])

                ob = outp.tile([112, 4, C], F32, tag="ob")
                for s in range(4):
                    c0 = s * 112
                    f_ps = psF.tile([P, C], F32, tag="fp")
                    nc.tensor.matmul(f_ps[:112, :],
                                     attnT[:, c0:c0 + 112], akp_sb[:],
                                     start=True, stop=True)
                    if s % 2 == 0:
                        nc.vector.tensor_copy(ob[:, s, :], f_ps[:112, :])
                    else:
                        nc.scalar.copy(ob[:, s, :], f_ps[:112, :])
                # one merged store per chunk: row n0+s*128+p <- ob[p, s, :]
                if stage >= 100:
                    dst = out[b, t * NCHUNK:(t + 1) * NCHUNK, :].rearrange(
                        "(s p) c -> p s c", s=4)
                    nc.scalar.dma_start(dst, ob[:])

    # Bacc.finalize -> compile(): alloc_regs + generate_event_semaphores
    # (multi-wait split). The PJRT run path does not finalize for us.
    nc.finalize()
    return nc


_CACHE = {}


def _get_nc():
    if "nc" not in _CACHE:
        _CACHE["nc"] = build_nc()
    return _CACHE["nc"]


def _make_const_inputs(Wq, Wkv, w_g, Wp, bp):
    Wq = np.ascontiguousarray(np.asarray(Wq, np.float32))
    Wkv = np.ascontiguousarray(np.asarray(Wkv, np.float32))
    Wp = np.ascontiguousarray(np.asarray(Wp, np.float32))
    w_g = np.ascontiguousarray(np.asarray(w_g, np.float32))
    bp = np.ascontiguousarray(np.asarray(bp, np.float32))

    def chunked(m):  # [CH*P, X] -> [P, CH*X] partition-major layout
        x = m.reshape(CH, P, -1).transpose(1, 0, 2)
        return np.ascontiguousarray(x).reshape(P, -1)

    hsel = np.zeros((HA, HEADS), np.float32)
    hsel[np.arange(HA), np.arange(HA) // A] = 1.0
    bdmask = np.zeros((C, HA), np.float32)
    for c in range(C):
        h = c // D
        bdmask[c, h * A:(h + 1) * A] = 1.0

    fparts = {
        "wqt": chunked(Wq.T / POOLN),
        "wkvt": chunked(Wkv.T / POOLN),
        "wg": chunked(w_g),
        "bpv": chunked(bp.reshape(C, 1)),
        "bdmask": chunked(bdmask),
    }
    blobf = np.concatenate([fparts[n] for n, _ in _F32_SECTS], axis=1)

    rp = {"wpt": chunked(Wp.T)}
    z = np.zeros((P, C), np.float32)
    z[0] = bp / HEADS
    rp["bp8"] = z
    rp["hsel"] = hsel
    z = np.zeros((P, HA), np.float32)
    z[:HEADS] = hsel.T
    rp["hselt"] = z
    z = np.zeros((P, HA), np.float32)
    z[0] = 1.0
    rp["ones1"] = z
    blobr = np.concatenate([rp[n] for n, _ in _F32R_SECTS], axis=1)

    blob16 = chunked(Wq).astype(ml_dtypes.bfloat16)
    return {"blob16": blob16, "blobf": blobf, "blobr": blobr}


def kernel(x, H=56, W=56, Wq=None, Wkv=None, w_g=None, Wp=None, bp=None,
           _trace=False, _trace_kwargs=None):
    x = np.asarray(x)
    assert x.shape == (B_FULL, N, C), x.shape
    x16 = np.asarray(x, np.float32).astype(ml_dtypes.bfloat16)

    consts = _make_const_inputs(Wq, Wkv, w_g, Wp, bp)
    in_maps = []
    for c in range(NCORES):
        m = dict(consts)
        m["x16"] = np.ascontiguousarray(x16[c * BPC:(c + 1) * BPC])
        in_maps.append(m)

    nc = _get_nc()
    res = run_bass_kernel_spmd(nc, in_maps, list(range(NCORES)),
                               trace=_trace, **(_trace_kwargs or {}))
    outs = np.concatenate([res.results[c]["out"] for c in range(NCORES)],
                          axis=0)
    if _trace:
        return outs.astype(np.float32, copy=False), res
    return outs.astype(np.float32, copy=False)
